# revision 31
# baseline (speedup 1.0000x reference)
"""Trainium2 Bass kernel for the NT-Xent / CLIP-style contrastive loss.

Reference computation (N=8192, D=512, fp32):
    zi_n, zj_n = row-normalize(z_i), row-normalize(z_j)
    sim = zi_n @ zj_n.T / TAU
    loss_e2t = mean_i( logsumexp_{j!=i}(sim[i,:]) - sim[i,i] )
    loss_t2e = mean_j( logsumexp_{i!=j}(sim[:,j]) - sim[j,j] )
    out = [ (loss_e2t+loss_t2e)/2, loss_e2t, loss_t2e ]

Sharding: rows of z_i are split across the 8 cores (1024 rows each); the
normalized z_j is replicated (the host plays the role of the all-gather).
Each core computes its [1024, 8192] tile of exp(sim), reducing it two ways:
  * row sums — fused into the ScalarE `activation(Exp, accum_out=...)`
  * col sums — partial per 128-partition group, accumulated on VectorE
    into a [128, 8192] buffer (the remaining 128-way + 8-core reduction
    is the host-side all-reduce)
The diagonal is NOT masked on device: since z_i != z_j the diagonal sims
are not outliers, so the host subtracts exp(pos) from the gathered sums
and finishes with log / means in float64.

Main matmul runs in fp8e4m3 with DoubleRow packing. Operands are scaled by
32 before the fp8 cast to stay clear of denormals; the 1/32^2 is folded
into the exp scale.

The scalar (ACT) engine is the steady-state bottleneck: 32 tiles x
(2048+352)/1.2GHz of Exp. To relieve it, one tile per column group (rc=7)
computes exp on the Vector engine instead, via the Schraudolph bit trick:
  exp(s) ~ bitcast_f32(int32(A*s + B)),  A = 2^23*log2(e)*exp_scale
one fused tensor_scalar (mult+add, fp32 PSUM -> int32 SBUF). B is tuned so
the relative error of SUMS of exps is ~2e-4 -- far inside the loss
tolerance. Row sums for those tiles come from a VectorE reduce.
"""

import os
import sys

for _p in ("/opt/trn_rl_repo", "/root/.axon_site/_ro/trn_rl_repo"):
    if os.path.isdir(_p) and _p not in sys.path:
        sys.path.insert(0, _p)

import numpy as np
import ml_dtypes

import concourse.bass as bass
import concourse.bacc as bacc
import concourse.mybir as mybir
import concourse.tile as tile
from concourse import bass_utils

TAU = 0.07
EPS = 1e-8

N = 8192            # batch
D = 512             # embed dim
NCORES = 8
NI = N // NCORES    # rows per core (1024)
P = 128             # partitions
RC = NI // P        # row chunks per core (8)
CCG = 2048          # columns per exp/accumulate group
NCCG = N // CCG     # 4 groups
MMN = 512           # matmul moving size (one PSUM bank of fp32)

DT_MAIN = os.environ.get("KERNEL_DT", "fp8")  # "fp8" | "bf16"
FP8_SCALE = 32.0
# Row-chunks whose tiles are SPLIT between ScalarE and VectorE: the ACT
# exps columns 0:SPLIT_W, a VectorE Schraudolph exps SPLIT_W:CCG. Scalar
# stays in the PSUM rotation on every tile (no skip bubbles) while ~40%
# of those tiles' exp work moves off the bottleneck engine.
SPLIT_RCS = () if os.environ.get("KERNEL_SCHRAUD", "1") == "0" else (2, 5)
SPLIT_W = 1024

BF16 = mybir.dt.bfloat16
F32 = mybir.dt.float32
I32 = mybir.dt.int32
FP8 = mybir.dt.float8e4
NP_FP8 = mybir.dt.np(FP8)

# Schraudolph constants: exp(x*exp_scale) ~ bitcast(int32(A*x + B)).
# C=485000 zeroes the mean relative error of the sawtooth (calibrated in
# numpy against exact exp over the actual sim distribution).
LOG2E = 1.4426950408889634
SCHRAUD_C = 485000.0

LAST_RESULTS = None  # BassKernelResults of the most recent run (for test.py)

_compiled = {}


def _build():
    """Build + compile the single-core SPMD Bass program."""
    nc = bacc.Bacc("TRN2", target_bir_lowering=False, debug=False)

    if DT_MAIN == "fp8":
        # zi: [kk, p, slab, n] with contraction row d = kk*256 + slab*128 + p.
        # zj adds a group dim so each [g] chunk is contiguous per partition
        # (16KB runs -> full DMA bandwidth): [kk, g, p, slab, cols-in-group].
        # Group 0 lives in its own tensor, pre-chunked per 512 columns and
        # contiguous per chunk, so the first matmuls can start on the head
        # of the stream without paying a strided (slow) DMA.
        zi_t = nc.dram_tensor("zi_t", [2, P, 2, NI], FP8, kind="ExternalInput")
        # group 0 is split into four source-AND-dest contiguous 512-col
        # sub-tensors so the first matmuls overlap the transfer tail
        zj0_t = nc.dram_tensor(
            "zj0_t", [2, CCG // MMN, P, 2, MMN], FP8, kind="ExternalInput"
        )
        zj_t = nc.dram_tensor(
            "zj_t", [2, NCCG - 1, P, 2, CCG], FP8, kind="ExternalInput"
        )
    else:
        zi_t = nc.dram_tensor("zi_t", [D, NI], BF16, kind="ExternalInput")
        zj_t = nc.dram_tensor("zj_t", [D, N], BF16, kind="ExternalInput")
    rows_d = nc.dram_tensor("rowsums", [P, RC * NCCG], F32, kind="ExternalOutput")
    cols_d = nc.dram_tensor("colacc", [P, N], BF16, kind="ExternalOutput")

    rows2_d = None
    if DT_MAIN == "fp8" and SPLIT_RCS:
        rows2_d = nc.dram_tensor(
            "rowsums2", [P, len(SPLIT_RCS) * NCCG], F32, kind="ExternalOutput"
        )

    with tile.TileContext(nc) as tc:
        _body(
            nc,
            tc,
            zi_t.ap(),
            zj0_t.ap() if DT_MAIN == "fp8" else None,
            zj_t.ap(),
            rows_d.ap(),
            rows2_d.ap() if rows2_d is not None else None,
            cols_d.ap(),
        )

    nc.compile()
    return nc


def _body(nc, tc, zi_t, zj0_t, zj_t, rows_d, rows2_d, cols_d):
    from contextlib import ExitStack

    fp8 = DT_MAIN == "fp8"
    kc = 2 if fp8 else 4  # contraction instruction count per output element
    exp_scale = 1.0 / (TAU * FP8_SCALE * FP8_SCALE) if fp8 else 1.0 / TAU
    perf_mode = mybir.MatmulPerfMode.DoubleRow if fp8 else None
    split_rcs = SPLIT_RCS if fp8 else ()

    schraud_a = float((2.0**23) * LOG2E * exp_scale)
    schraud_b = float(127.0 * 2.0**23 - SCHRAUD_C)

    with ExitStack() as ctx:
        zpool = ctx.enter_context(tc.tile_pool(name="z", bufs=1))
        epool = ctx.enter_context(tc.tile_pool(name="e", bufs=6))
        ipool = ctx.enter_context(tc.tile_pool(name="ei", bufs=3))
        apool = ctx.enter_context(tc.tile_pool(name="acc", bufs=1))
        psump = ctx.enter_context(
            tc.tile_pool(name="psum", bufs=2, space=bass.MemorySpace.PSUM)
        )

        colacc = apool.tile([P, N], BF16, tag="colacc")
        rows_sb = apool.tile([P, RC * NCCG], F32, tag="rows")

        # ---- PE clock warmup + early Exp table load -------------------
        # A few dummy DoubleRow matmuls on a memset tile keep the PE busy
        # during the input DMA window so the HAM clock gate opens (1.2 ->
        # 2.4 GHz) before the first real matmul issues. The zero-scale
        # activation pulls the ~1.3us Exp ACT_TABLE_LOAD off the first
        # real tile's critical path. The memset is the first gpsimd
        # instruction so the warmup chain starts right after iram load.
        if fp8:
            wsrc = zpool.tile([P, 2, MMN], FP8, tag="wsrc", name="wsrc")
            nc.gpsimd.memset(wsrc[:, :, 0:P], 0)
            dummy = epool.tile([P, 8], BF16, tag="dummy", name="dummy")
            nc.scalar.activation(
                dummy[:],
                wsrc[:, 0, 0:8],
                mybir.ActivationFunctionType.Exp,
                bias=0.0,
                scale=0.0,
            )
            wp = psump.tile([P, MMN], F32, tag="G", name="warm")
            for w in range(7):
                nc.tensor.matmul(
                    wp[:],
                    wsrc[:, :, 0:P],
                    wsrc[:],
                    start=True,
                    stop=True,
                    perf_mode=perf_mode,
                )

        # ---- stage inputs in SBUF -------------------------------------
        # Two HWDGE rings (sync + scalar) stream the two contraction
        # halves in parallel; the last zj groups for the scalar ring move
        # to the gpsimd SWDGE queue so the scalar queue frees up early.
        # zj lives in per-(k, group) tiles so every group transfer writes
        # a fully contiguous 4KB-per-partition destination (small strided
        # runs cut the ring to ~1/3 rate).
        if fp8:
            zi_sb = [
                zpool.tile([P, 2, NI], FP8, tag=f"zi{k}", name=f"zi{k}")
                for k in range(kc)
            ]
            # group 0: four source/dest-contiguous sub-tiles per k half so
            # the first matmuls start on the head of the stream; groups
            # 1..3: one contiguous tile per (k, g).
            zj0_sb = [
                [
                    zpool.tile(
                        [P, 2, MMN], FP8, tag=f"zj0k{k}c{cc}", name=f"zj0k{k}c{cc}"
                    )
                    for cc in range(CCG // MMN)
                ]
                for k in range(kc)
            ]
            zj_sb = [
                [None]
                + [
                    zpool.tile([P, 2, CCG], FP8, tag=f"zj{k}g{g}", name=f"zj{k}g{g}")
                    for g in range(1, NCCG)
                ]
                for k in range(kc)
            ]

            def _zj(eng, k, g):
                eng.dma_start(zj_sb[k][g][:], zj_t[k, g - 1, :, :, :])

            # iter 0 only needs zi cols 0:128 — land those first, then the
            # first column group per 512-col chunk, then the rest in
            # consumption order. g2k1/g3k1 ride the gpsimd SWDGE queue but
            # are issued later (inside the main loop) so they do not steal
            # HBM bandwidth from the critical group-0 window.
            nc.sync.dma_start(zi_sb[0][:, :, 0:P], zi_t[0, :, :, 0:P])
            nc.scalar.dma_start(zi_sb[1][:, :, 0:P], zi_t[1, :, :, 0:P])
            for cc in range(CCG // MMN):
                nc.sync.dma_start(zj0_sb[0][cc][:], zj0_t[0, cc, :, :, :])
                nc.scalar.dma_start(zj0_sb[1][cc][:], zj0_t[1, cc, :, :, :])
            nc.sync.dma_start(zi_sb[0][:, :, P:NI], zi_t[0, :, :, P:NI])
            nc.scalar.dma_start(zi_sb[1][:, :, P:NI], zi_t[1, :, :, P:NI])
            _zj(nc.sync, 0, 1)
            _zj(nc.scalar, 1, 1)
            _zj(nc.sync, 0, 2)
            _zj(nc.sync, 0, 3)
        else:
            zi_sb = [
                zpool.tile([P, NI], BF16, tag=f"zi{k}", name=f"zi{k}")
                for k in range(kc)
            ]
            zj_sb = [
                zpool.tile([P, N], BF16, tag=f"zj{k}", name=f"zj{k}")
                for k in range(kc)
            ]
            for k in range(kc):
                nc.sync.dma_start(zi_sb[k][:], zi_t[k * P:(k + 1) * P, :])
            for g in range(NCCG):
                c0, c1 = g * CCG, (g + 1) * CCG
                for k in range(kc):
                    nc.sync.dma_start(
                        zj_sb[k][:, c0:c1], zj_t[k * P:(k + 1) * P, c0:c1]
                    )

        # ---- main loop ------------------------------------------------
        # VectorE ops are emitted through a pending list: at every split
        # tile the PSUM-freeing Schraudolph tensor_scalar is emitted FIRST
        # and the backlog of ACT-gated colacc adds after it, so the DVE
        # queue order matches dependency-readiness order and the PSUM
        # rotation never stalls behind a gated add.
        rows2_sb = None
        if split_rcs:
            rows2_sb = apool.tile([P, len(split_rcs) * NCCG], F32, tag="rows2")
        pending = []

        def flush():
            for op in pending:
                op()
            pending.clear()

        for g in range(NCCG):
            c0 = g * CCG
            for rc in range(RC):
                gp = psump.tile([P, CCG], F32, tag="G")
                for k in range(kc):
                    if fp8:
                        lhsT = zi_sb[k][:, :, rc * P:(rc + 1) * P]
                    else:
                        lhsT = zi_sb[k][:, rc * P:(rc + 1) * P]
                    for cc in range(CCG // MMN):
                        if fp8:
                            if g == 0:
                                rhs = zj0_sb[k][cc][:]
                            else:
                                rhs = zj_sb[k][g][:, :, cc * MMN:(cc + 1) * MMN]
                        else:
                            rhs = zj_sb[k][:, c0 + cc * MMN:c0 + (cc + 1) * MMN]
                        nc.tensor.matmul(
                            gp[:, cc * MMN:(cc + 1) * MMN],
                            lhsT,
                            rhs,
                            start=(k == 0),
                            stop=(k == kc - 1),
                            perf_mode=perf_mode,
                        )
                rcol = rows_sb[:, rc * NCCG + g:rc * NCCG + g + 1]
                if rc in split_rcs:
                    # split tile: ACT exps the first SPLIT_W columns, a
                    # VectorE Schraudolph (fused mult+add, int32 out whose
                    # bit pattern IS exp(sim/tau)) exps the rest. Scalar
                    # never leaves the PSUM rotation, so no skip bubble.
                    w = SPLIT_W
                    sidx = split_rcs.index(rc) * NCCG + g
                    et = epool.tile([P, CCG], BF16, tag="E")
                    nc.scalar.activation(
                        et[:, 0:w],
                        gp[:, 0:w],
                        mybir.ActivationFunctionType.Exp,
                        bias=0.0,
                        scale=exp_scale,
                        accum_out=rcol,
                    )
                    eti = ipool.tile([P, CCG - SPLIT_W], I32, tag="EI")
                    nc.vector.tensor_scalar(
                        eti[:],
                        gp[:, w:CCG],
                        schraud_a,
                        schraud_b,
                        mybir.AluOpType.mult,
                        mybir.AluOpType.add,
                    )
                    flush()
                    etf = eti[:].bitcast(F32)

                    def split_ops(et=et, etf=etf, c0=c0, w=w, sidx=sidx):
                        nc.vector.tensor_add(
                            colacc[:, c0:c0 + w], colacc[:, c0:c0 + w], et[:, 0:w]
                        )
                        nc.vector.tensor_tensor(
                            colacc[:, c0 + w:c0 + CCG],
                            colacc[:, c0 + w:c0 + CCG],
                            etf,
                            mybir.AluOpType.add,
                        )
                        nc.vector.tensor_reduce(
                            rows2_sb[:, sidx:sidx + 1],
                            etf,
                            mybir.AxisListType.X,
                            mybir.AluOpType.add,
                        )

                    pending.append(split_ops)
                elif rc == 0:
                    # first row chunk of a group: the ACT writes straight
                    # into colacc — no VectorE copy needed.
                    nc.scalar.activation(
                        colacc[:, c0:c0 + CCG],
                        gp[:],
                        mybir.ActivationFunctionType.Exp,
                        bias=0.0,
                        scale=exp_scale,
                        accum_out=rcol,
                    )
                else:
                    et = epool.tile([P, CCG], BF16, tag="E")
                    nc.scalar.activation(
                        et[:],
                        gp[:],
                        mybir.ActivationFunctionType.Exp,
                        bias=0.0,
                        scale=exp_scale,
                        accum_out=rcol,
                    )

                    def add_op(et=et, c0=c0):
                        nc.vector.tensor_add(
                            colacc[:, c0:c0 + CCG], colacc[:, c0:c0 + CCG], et[:]
                        )

                    pending.append(add_op)
                if fp8 and g == 0 and rc == 3:
                    # late-issued SWDGE transfers: a tiny gpsimd copy gated
                    # on this tile's ACT keeps them out of the group-0
                    # bandwidth window.
                    gsc = apool.tile([P, 8], BF16, tag="gsc")
                    nc.gpsimd.tensor_copy(gsc[:], et[:, 0:8])
                    nc.gpsimd.dma_start(
                        zj_sb[1][2][:], zj_t[1, 1, :, :, :]
                    )
                    nc.gpsimd.dma_start(
                        zj_sb[1][3][:], zj_t[1, 2, :, :, :]
                    )
            flush()
            # this group's columns are done — ship them while the next
            # runs; the final group splits across both rings to halve the
            # drain tail.
            if g < NCCG - 1:
                nc.sync.dma_start(cols_d[:, c0:c0 + CCG], colacc[:, c0:c0 + CCG])
            else:
                h = CCG // 2
                nc.sync.dma_start(cols_d[:, c0:c0 + h], colacc[:, c0:c0 + h])
                nc.scalar.dma_start(
                    cols_d[:, c0 + h:c0 + CCG], colacc[:, c0 + h:c0 + CCG]
                )

        nc.scalar.dma_start(rows_d[:, :], rows_sb[:])
        if split_rcs:
            nc.scalar.dma_start(rows2_d[:, :], rows2_sb[:])


def _get_nc():
    if "nc" not in _compiled:
        _compiled["nc"] = _build()
    return _compiled["nc"]


def _pack_fp8(zt):
    """[D, n] fp32 -> [2, 128, 2, n] fp8 with d = kk*256 + slab*128 + p."""
    n = zt.shape[1]
    return np.ascontiguousarray(
        (zt * FP8_SCALE).reshape(2, 2, P, n).transpose(0, 2, 1, 3)
    ).astype(NP_FP8)


def _pack_fp8_zj(zt):
    """[D, N] fp32 -> (group-0 pack [2, 4, 128, 2, 512], groups-1.. pack
    [2, NCCG-1, 128, 2, CCG]) fp8 with d = kk*256 + slab*128 + p. Every
    DMA source chunk is contiguous for full-rate transfers."""
    q = (zt * FP8_SCALE).reshape(2, 2, P, NCCG, CCG)
    g0 = q[:, :, :, 0, :].reshape(2, 2, P, CCG // MMN, MMN)
    zj0 = np.ascontiguousarray(g0.transpose(0, 3, 2, 1, 4)).astype(NP_FP8)
    rest = np.ascontiguousarray(
        q[:, :, :, 1:, :].transpose(0, 3, 2, 1, 4)
    ).astype(NP_FP8)
    return zj0, rest


def _prep_inputs(z_i, z_j):
    """Host-side sharding: normalize (fp32, as the reference), transpose to
    [D, N] (the layout the PE contracts over), quantize, slice per core."""
    zi = np.asarray(z_i, dtype=np.float32)
    zj = np.asarray(z_j, dtype=np.float32)
    ni = np.maximum(np.sqrt((zi * zi).sum(-1, keepdims=True)), EPS)
    nj = np.maximum(np.sqrt((zj * zj).sum(-1, keepdims=True)), EPS)
    zin = zi / ni
    zjn = zj / nj
    pos = (zin * zjn).sum(-1, dtype=np.float64) / TAU  # diagonal of sim, [N]

    zin_t = zin.T  # [D, N]
    zjn_t = zjn.T

    in_maps = []
    if DT_MAIN == "fp8":
        zj0_pack, zj_pack = _pack_fp8_zj(zjn_t)
        for c in range(NCORES):
            in_maps.append(
                {
                    "zi_t": _pack_fp8(zin_t[:, c * NI:(c + 1) * NI]),
                    "zj0_t": zj0_pack,
                    "zj_t": zj_pack,
                }
            )
    else:
        zin_b = np.ascontiguousarray(zin_t.astype(ml_dtypes.bfloat16))
        zjn_b = np.ascontiguousarray(zjn_t.astype(ml_dtypes.bfloat16))
        for c in range(NCORES):
            in_maps.append(
                {
                    "zi_t": np.ascontiguousarray(zin_b[:, c * NI:(c + 1) * NI]),
                    "zj_t": zjn_b,
                }
            )
    return in_maps, pos


def kernel(z_i, z_j):
    global LAST_RESULTS
    in_maps, pos = _prep_inputs(z_i, z_j)
    nc = _get_nc()

    res = bass_utils.run_bass_kernel_spmd(nc, in_maps, core_ids=list(range(NCORES)))
    LAST_RESULTS = res

    split_rcs = SPLIT_RCS if DT_MAIN == "fp8" else ()
    rowsum = np.zeros(N, dtype=np.float64)
    colsum = np.zeros(N, dtype=np.float64)
    for c in range(NCORES):
        out = res.results[c]
        rs = out["rowsums"].astype(np.float64)  # [128, RC*NCCG]
        # column rc*NCCG+g holds sum over group g's 2048 cols for row chunk rc
        rs = rs.reshape(P, RC, NCCG)            # [p, rc, g]
        if split_rcs:
            # split tiles keep their VectorE half's row sums in rowsums2
            rs2 = out["rowsums2"].astype(np.float64).reshape(
                P, len(split_rcs), NCCG
            )
            for si, rc in enumerate(split_rcs):
                rs[:, rc, :] += rs2[:, si, :]
        rs = rs.sum(-1)                         # [p, rc]
        rowsum[c * NI:(c + 1) * NI] = rs.T.reshape(-1)  # global row = rc*128+p
        colsum += out["colacc"].astype(np.float64).sum(0)

    # host-side "all-reduce" epilogue: drop the diagonal, logs, means
    exp_pos = np.exp(pos)
    lse_row = np.log(rowsum - exp_pos)
    lse_col = np.log(colsum - exp_pos)
    loss_e2t = np.mean(lse_row - pos)
    loss_t2e = np.mean(lse_col - pos)
    loss = 0.5 * (loss_e2t + loss_t2e)
    return np.stack([loss, loss_e2t, loss_t2e]).astype(np.float32)


# revision 40
# speedup vs baseline: 1.0375x; 1.0375x over previous
"""Trainium2 Bass kernel for the NT-Xent / CLIP-style contrastive loss.

Reference computation (N=8192, D=512, fp32):
    zi_n, zj_n = row-normalize(z_i), row-normalize(z_j)
    sim = zi_n @ zj_n.T / TAU
    loss_e2t = mean_i( logsumexp_{j!=i}(sim[i,:]) - sim[i,i] )
    loss_t2e = mean_j( logsumexp_{i!=j}(sim[:,j]) - sim[j,j] )
    out = [ (loss_e2t+loss_t2e)/2, loss_e2t, loss_t2e ]

Sharding: rows of z_i are split across the 8 cores (1024 rows each); the
normalized z_j is replicated (the host plays the role of the all-gather).
Each core computes its [1024, 8192] tile of exp(sim), reducing it two ways:
  * row sums — fused into the ScalarE `activation(Exp, accum_out=...)`
  * col sums — partial per 128-partition group, accumulated on VectorE
    into a [128, 8192] buffer (the remaining 128-way + 8-core reduction
    is the host-side all-reduce)
The diagonal is NOT masked on device: since z_i != z_j the diagonal sims
are not outliers, so the host subtracts exp(pos) from the gathered sums
and finishes with log / means in float64.

Main matmul runs in fp8e4m3 with DoubleRow packing. Operands are scaled by
32 before the fp8 cast to stay clear of denormals; the 1/32^2 is folded
into the exp scale.

The scalar (ACT) engine is the steady-state bottleneck: 32 tiles x
(2048+352)/1.2GHz of Exp. To relieve it, one tile per column group (rc=7)
computes exp on the Vector engine instead, via the Schraudolph bit trick:
  exp(s) ~ bitcast_f32(int32(A*s + B)),  A = 2^23*log2(e)*exp_scale
one fused tensor_scalar (mult+add, fp32 PSUM -> int32 SBUF). B is tuned so
the relative error of SUMS of exps is ~2e-4 -- far inside the loss
tolerance. Row sums for those tiles come from a VectorE reduce.
"""

import os
import sys

for _p in ("/opt/trn_rl_repo", "/root/.axon_site/_ro/trn_rl_repo"):
    if os.path.isdir(_p) and _p not in sys.path:
        sys.path.insert(0, _p)

import numpy as np
import ml_dtypes

import concourse.bass as bass
import concourse.bacc as bacc
import concourse.mybir as mybir
import concourse.tile as tile
from concourse import bass_utils

TAU = 0.07
EPS = 1e-8

N = 8192            # batch
D = 512             # embed dim
NCORES = 8
NI = N // NCORES    # rows per core (1024)
P = 128             # partitions
RC = NI // P        # row chunks per core (8)
CCG = 2048          # columns per exp/accumulate group
NCCG = N // CCG     # 4 groups
MMN = 512           # matmul moving size (one PSUM bank of fp32)

DT_MAIN = os.environ.get("KERNEL_DT", "fp8")  # "fp8" | "bf16"
FP8_SCALE = 32.0
# Row-chunks whose tiles are SPLIT between ScalarE and VectorE: the ACT
# exps columns 0:SPLIT_W, a VectorE Schraudolph exps SPLIT_W:CCG. Scalar
# stays in the PSUM rotation on every tile (no skip bubbles) while ~40%
# of those tiles' exp work moves off the bottleneck engine.
SPLIT_RCS = () if os.environ.get("KERNEL_SCHRAUD", "1") == "0" else (2, 5)
SPLIT_W = 1024

BF16 = mybir.dt.bfloat16
F32 = mybir.dt.float32
I32 = mybir.dt.int32
FP8 = mybir.dt.float8e4
NP_FP8 = mybir.dt.np(FP8)

# Schraudolph constants: exp(x*exp_scale) ~ bitcast(int32(A*x + B)).
# C=485000 zeroes the mean relative error of the sawtooth (calibrated in
# numpy against exact exp over the actual sim distribution).
LOG2E = 1.4426950408889634
SCHRAUD_C = 485000.0

LAST_RESULTS = None  # BassKernelResults of the most recent run (for test.py)

_compiled = {}


def _build():
    """Build + compile the single-core SPMD Bass program."""
    nc = bacc.Bacc("TRN2", target_bir_lowering=False, debug=False)

    if DT_MAIN == "fp8":
        # zi: [kk, p, slab, n] with contraction row d = kk*256 + slab*128 + p.
        # zj adds a group dim so each [g] chunk is contiguous per partition
        # (16KB runs -> full DMA bandwidth): [kk, g, p, slab, cols-in-group].
        # Group 0 lives in its own tensor, pre-chunked per 512 columns and
        # contiguous per chunk, so the first matmuls can start on the head
        # of the stream without paying a strided (slow) DMA.
        zi_t = nc.dram_tensor("zi_t", [2, P, 2, NI], FP8, kind="ExternalInput")
        zj_t = nc.dram_tensor(
            "zj_t", [2, NCCG, P, 2, CCG], FP8, kind="ExternalInput"
        )
    else:
        zi_t = nc.dram_tensor("zi_t", [D, NI], BF16, kind="ExternalInput")
        zj_t = nc.dram_tensor("zj_t", [D, N], BF16, kind="ExternalInput")
    rows_d = nc.dram_tensor("rowsums", [P, RC * NCCG], F32, kind="ExternalOutput")
    cols_d = nc.dram_tensor("colacc", [P, N], BF16, kind="ExternalOutput")

    rows2_d = None
    if DT_MAIN == "fp8" and SPLIT_RCS:
        rows2_d = nc.dram_tensor(
            "rowsums2", [P, len(SPLIT_RCS) * NCCG], F32, kind="ExternalOutput"
        )

    with tile.TileContext(nc) as tc:
        _body(
            nc,
            tc,
            zi_t.ap(),
            zj_t.ap(),
            rows_d.ap(),
            rows2_d.ap() if rows2_d is not None else None,
            cols_d.ap(),
        )

    nc.compile()
    return nc


def _body(nc, tc, zi_t, zj_t, rows_d, rows2_d, cols_d):
    from contextlib import ExitStack

    fp8 = DT_MAIN == "fp8"
    kc = 2 if fp8 else 4  # contraction instruction count per output element
    exp_scale = 1.0 / (TAU * FP8_SCALE * FP8_SCALE) if fp8 else 1.0 / TAU
    perf_mode = mybir.MatmulPerfMode.DoubleRow if fp8 else None
    split_rcs = SPLIT_RCS if fp8 else ()

    schraud_a = float((2.0**23) * LOG2E * exp_scale)
    schraud_b = float(127.0 * 2.0**23 - SCHRAUD_C)

    with ExitStack() as ctx:
        zpool = ctx.enter_context(tc.tile_pool(name="z", bufs=1))
        epool = ctx.enter_context(tc.tile_pool(name="e", bufs=6))
        ipool = ctx.enter_context(tc.tile_pool(name="ei", bufs=3))
        apool = ctx.enter_context(tc.tile_pool(name="acc", bufs=1))
        psump = ctx.enter_context(
            tc.tile_pool(name="psum", bufs=2, space=bass.MemorySpace.PSUM)
        )

        colacc = apool.tile([P, N], BF16, tag="colacc")
        rows_sb = apool.tile([P, RC * NCCG], F32, tag="rows")

        # ---- PE clock warmup + early Exp table load -------------------
        # A few dummy DoubleRow matmuls on a memset tile keep the PE busy
        # during the input DMA window so the HAM clock gate opens (1.2 ->
        # 2.4 GHz) before the first real matmul issues. The zero-scale
        # activation pulls the ~1.3us Exp ACT_TABLE_LOAD off the first
        # real tile's critical path. The memset is the first gpsimd
        # instruction so the warmup chain starts right after iram load.
        if fp8:
            wsrc = zpool.tile([P, 2, MMN], FP8, tag="wsrc", name="wsrc")
            nc.gpsimd.memset(wsrc[:, :, 0:P], 0)
            dummy = epool.tile([P, 8], BF16, tag="dummy", name="dummy")
            nc.scalar.activation(
                dummy[:],
                wsrc[:, 0, 0:8],
                mybir.ActivationFunctionType.Exp,
                bias=0.0,
                scale=0.0,
            )
            wp = psump.tile([P, MMN], F32, tag="G", name="warm")
            for w in range(7):
                nc.tensor.matmul(
                    wp[:],
                    wsrc[:, :, 0:P],
                    wsrc[:],
                    start=True,
                    stop=True,
                    perf_mode=perf_mode,
                )

        # ---- stage inputs in SBUF -------------------------------------
        # Two HWDGE rings (sync + scalar) stream the two contraction
        # halves in parallel; the last zj groups for the scalar ring move
        # to the gpsimd SWDGE queue so the scalar queue frees up early.
        # zj lives in per-(k, group) tiles so every group transfer writes
        # a fully contiguous 4KB-per-partition destination (small strided
        # runs cut the ring to ~1/3 rate).
        if fp8:
            zi_sb = [
                zpool.tile([P, 2, NI], FP8, tag=f"zi{k}", name=f"zi{k}")
                for k in range(kc)
            ]
            zj_sb = [
                [
                    zpool.tile([P, 2, CCG], FP8, tag=f"zj{k}g{g}", name=f"zj{k}g{g}")
                    for g in range(NCCG)
                ]
                for k in range(kc)
            ]

            def _zj(eng, k, g):
                eng.dma_start(zj_sb[k][g][:], zj_t[k, g, :, :, :])

            # Each ring transfer pays ~2us fixed latency, so the critical
            # path keeps transfer COUNT minimal: a small zi head (cols
            # 0:128, all iter-0 matmuls need) then the whole first group,
            # then everything else in consumption order.
            nc.sync.dma_start(zi_sb[0][:, :, 0:P], zi_t[0, :, :, 0:P])
            nc.scalar.dma_start(zi_sb[1][:, :, 0:P], zi_t[1, :, :, 0:P])
            _zj(nc.sync, 0, 0)
            _zj(nc.scalar, 1, 0)
            nc.sync.dma_start(zi_sb[0][:, :, P:NI], zi_t[0, :, :, P:NI])
            nc.scalar.dma_start(zi_sb[1][:, :, P:NI], zi_t[1, :, :, P:NI])
            _zj(nc.sync, 0, 1)
            _zj(nc.scalar, 1, 1)
            _zj(nc.sync, 0, 2)
            _zj(nc.scalar, 1, 2)
            _zj(nc.sync, 0, 3)
            _zj(nc.scalar, 1, 3)
        else:
            zi_sb = [
                zpool.tile([P, NI], BF16, tag=f"zi{k}", name=f"zi{k}")
                for k in range(kc)
            ]
            zj_sb = [
                zpool.tile([P, N], BF16, tag=f"zj{k}", name=f"zj{k}")
                for k in range(kc)
            ]
            for k in range(kc):
                nc.sync.dma_start(zi_sb[k][:], zi_t[k * P:(k + 1) * P, :])
            for g in range(NCCG):
                c0, c1 = g * CCG, (g + 1) * CCG
                for k in range(kc):
                    nc.sync.dma_start(
                        zj_sb[k][:, c0:c1], zj_t[k * P:(k + 1) * P, c0:c1]
                    )

        # ---- main loop ------------------------------------------------
        # VectorE ops are emitted through a pending list: at every split
        # tile the PSUM-freeing Schraudolph tensor_scalar is emitted FIRST
        # and the backlog of ACT-gated colacc adds after it, so the DVE
        # queue order matches dependency-readiness order and the PSUM
        # rotation never stalls behind a gated add.
        rows2_sb = None
        if split_rcs:
            rows2_sb = apool.tile([P, len(split_rcs) * NCCG], F32, tag="rows2")
        pending = []

        def flush():
            for op in pending:
                op()
            pending.clear()

        for g in range(NCCG):
            c0 = g * CCG
            for rc in range(RC):
                gp = psump.tile([P, CCG], F32, tag="G")
                for k in range(kc):
                    if fp8:
                        lhsT = zi_sb[k][:, :, rc * P:(rc + 1) * P]
                    else:
                        lhsT = zi_sb[k][:, rc * P:(rc + 1) * P]
                    for cc in range(CCG // MMN):
                        if fp8:
                            rhs = zj_sb[k][g][:, :, cc * MMN:(cc + 1) * MMN]
                        else:
                            rhs = zj_sb[k][:, c0 + cc * MMN:c0 + (cc + 1) * MMN]
                        nc.tensor.matmul(
                            gp[:, cc * MMN:(cc + 1) * MMN],
                            lhsT,
                            rhs,
                            start=(k == 0),
                            stop=(k == kc - 1),
                            perf_mode=perf_mode,
                        )
                rcol = rows_sb[:, rc * NCCG + g:rc * NCCG + g + 1]
                if rc in split_rcs:
                    # split tile: ACT exps the first SPLIT_W columns, a
                    # VectorE Schraudolph (fused mult+add, int32 out whose
                    # bit pattern IS exp(sim/tau)) exps the rest. Scalar
                    # never leaves the PSUM rotation, so no skip bubble.
                    w = SPLIT_W
                    sidx = split_rcs.index(rc) * NCCG + g
                    et = epool.tile([P, CCG], BF16, tag="E")
                    nc.scalar.activation(
                        et[:, 0:w],
                        gp[:, 0:w],
                        mybir.ActivationFunctionType.Exp,
                        bias=0.0,
                        scale=exp_scale,
                        accum_out=rcol,
                    )
                    eti = ipool.tile([P, CCG - SPLIT_W], I32, tag="EI")
                    # the tensor_scalar frees this PSUM buf; scheduled as
                    # if issued ~2 tiles earlier so it lands ahead of the
                    # ACT-gated colacc adds in the Vector queue
                    with tc.high_priority(offset=26):
                        nc.vector.tensor_scalar(
                            eti[:],
                            gp[:, w:CCG],
                            schraud_a,
                            schraud_b,
                            mybir.AluOpType.mult,
                            mybir.AluOpType.add,
                        )
                    flush()
                    etf = eti[:].bitcast(F32)

                    def split_ops(et=et, etf=etf, c0=c0, w=w, sidx=sidx):
                        nc.vector.tensor_add(
                            colacc[:, c0:c0 + w], colacc[:, c0:c0 + w], et[:, 0:w]
                        )
                        nc.vector.tensor_tensor(
                            colacc[:, c0 + w:c0 + CCG],
                            colacc[:, c0 + w:c0 + CCG],
                            etf,
                            mybir.AluOpType.add,
                        )
                        nc.vector.tensor_reduce(
                            rows2_sb[:, sidx:sidx + 1],
                            etf,
                            mybir.AxisListType.X,
                            mybir.AluOpType.add,
                        )

                    pending.append(split_ops)
                elif rc == 0:
                    # first row chunk of a group: the ACT writes straight
                    # into colacc — no VectorE copy needed.
                    nc.scalar.activation(
                        colacc[:, c0:c0 + CCG],
                        gp[:],
                        mybir.ActivationFunctionType.Exp,
                        bias=0.0,
                        scale=exp_scale,
                        accum_out=rcol,
                    )
                else:
                    et = epool.tile([P, CCG], BF16, tag="E")
                    nc.scalar.activation(
                        et[:],
                        gp[:],
                        mybir.ActivationFunctionType.Exp,
                        bias=0.0,
                        scale=exp_scale,
                        accum_out=rcol,
                    )

                    def add_op(et=et, c0=c0):
                        nc.vector.tensor_add(
                            colacc[:, c0:c0 + CCG], colacc[:, c0:c0 + CCG], et[:]
                        )

                    pending.append(add_op)
            flush()
            # this group's columns are done — ship them while the next
            # runs; the final group splits across both rings to halve the
            # drain tail.
            if g < NCCG - 1:
                nc.sync.dma_start(cols_d[:, c0:c0 + CCG], colacc[:, c0:c0 + CCG])
            else:
                h = CCG // 2
                nc.sync.dma_start(cols_d[:, c0:c0 + h], colacc[:, c0:c0 + h])
                nc.scalar.dma_start(
                    cols_d[:, c0 + h:c0 + CCG], colacc[:, c0 + h:c0 + CCG]
                )

        nc.scalar.dma_start(rows_d[:, :], rows_sb[:])
        if split_rcs:
            nc.scalar.dma_start(rows2_d[:, :], rows2_sb[:])


def _get_nc():
    if "nc" not in _compiled:
        _compiled["nc"] = _build()
    return _compiled["nc"]


def _pack_fp8(zt):
    """[D, n] fp32 -> [2, 128, 2, n] fp8 with d = kk*256 + slab*128 + p."""
    n = zt.shape[1]
    return np.ascontiguousarray(
        (zt * FP8_SCALE).reshape(2, 2, P, n).transpose(0, 2, 1, 3)
    ).astype(NP_FP8)


def _pack_fp8_zj(zt):
    """[D, N] fp32 -> [2, NCCG, 128, 2, CCG] fp8: d = kk*256 + slab*128 + p,
    col = g*CCG + c. Each [kk, g] chunk is contiguous for full-rate DMA."""
    return np.ascontiguousarray(
        (zt * FP8_SCALE).reshape(2, 2, P, NCCG, CCG).transpose(0, 3, 2, 1, 4)
    ).astype(NP_FP8)


def _prep_inputs(z_i, z_j):
    """Host-side sharding: normalize (fp32, as the reference), transpose to
    [D, N] (the layout the PE contracts over), quantize, slice per core."""
    zi = np.asarray(z_i, dtype=np.float32)
    zj = np.asarray(z_j, dtype=np.float32)
    ni = np.maximum(np.sqrt((zi * zi).sum(-1, keepdims=True)), EPS)
    nj = np.maximum(np.sqrt((zj * zj).sum(-1, keepdims=True)), EPS)
    zin = zi / ni
    zjn = zj / nj
    pos = (zin * zjn).sum(-1, dtype=np.float64) / TAU  # diagonal of sim, [N]

    zin_t = zin.T  # [D, N]
    zjn_t = zjn.T

    in_maps = []
    if DT_MAIN == "fp8":
        zj_pack = _pack_fp8_zj(zjn_t)
        for c in range(NCORES):
            in_maps.append(
                {
                    "zi_t": _pack_fp8(zin_t[:, c * NI:(c + 1) * NI]),
                    "zj_t": zj_pack,
                }
            )
    else:
        zin_b = np.ascontiguousarray(zin_t.astype(ml_dtypes.bfloat16))
        zjn_b = np.ascontiguousarray(zjn_t.astype(ml_dtypes.bfloat16))
        for c in range(NCORES):
            in_maps.append(
                {
                    "zi_t": np.ascontiguousarray(zin_b[:, c * NI:(c + 1) * NI]),
                    "zj_t": zjn_b,
                }
            )
    return in_maps, pos


def kernel(z_i, z_j):
    global LAST_RESULTS
    in_maps, pos = _prep_inputs(z_i, z_j)
    nc = _get_nc()

    res = bass_utils.run_bass_kernel_spmd(nc, in_maps, core_ids=list(range(NCORES)))
    LAST_RESULTS = res

    split_rcs = SPLIT_RCS if DT_MAIN == "fp8" else ()
    rowsum = np.zeros(N, dtype=np.float64)
    colsum = np.zeros(N, dtype=np.float64)
    for c in range(NCORES):
        out = res.results[c]
        rs = out["rowsums"].astype(np.float64)  # [128, RC*NCCG]
        # column rc*NCCG+g holds sum over group g's 2048 cols for row chunk rc
        rs = rs.reshape(P, RC, NCCG)            # [p, rc, g]
        if split_rcs:
            # split tiles keep their VectorE half's row sums in rowsums2
            rs2 = out["rowsums2"].astype(np.float64).reshape(
                P, len(split_rcs), NCCG
            )
            for si, rc in enumerate(split_rcs):
                rs[:, rc, :] += rs2[:, si, :]
        rs = rs.sum(-1)                         # [p, rc]
        rowsum[c * NI:(c + 1) * NI] = rs.T.reshape(-1)  # global row = rc*128+p
        colsum += out["colacc"].astype(np.float64).sum(0)

    # host-side "all-reduce" epilogue: drop the diagonal, logs, means
    exp_pos = np.exp(pos)
    lse_row = np.log(rowsum - exp_pos)
    lse_col = np.log(colsum - exp_pos)
    loss_e2t = np.mean(lse_row - pos)
    loss_t2e = np.mean(lse_col - pos)
    loss = 0.5 * (loss_e2t + loss_t2e)
    return np.stack([loss, loss_e2t, loss_t2e]).astype(np.float32)
